# revision 9
# baseline (speedup 1.0000x reference)
"""GQA causal attention block (B=2, S=2048, D=1024, H=16, KVH=4) on 8 trn2
NeuronCores.

Sharding: core = (b, g) for batch b in {0,1} and kv-head group g in {0..3}.
Each core projects q for its 4 query heads (column-parallel wq), k/v for its
kv head, applies RoPE, runs causal attention for its 4 heads, and computes a
row-parallel partial of the output projection (its 256 rows of wo). The host
sums the 4 partials per batch.

Device layout choices:
- x is shipped pre-transposed (xt = x[b].T) so the D contraction sits on
  SBUF partitions for every projection matmul.
- RoPE head dims are de-interleaved (even dims then odd dims) via a host-side
  permutation of wq/wk columns, and the rotation partner comes from a second
  matmul against sign-flipped swapped columns:
      q_rot = cos * (x @ wq_deint) + sin * (x @ wq_swap)
  which keeps every vector op partition-aligned (the hardware requires all
  elementwise operands on identical partitions). Scores are invariant to the
  shared permutation of q and k head dims.
- q heads are produced in pair tiles (two heads stacked on 128 partitions);
  k is produced duplicated on both 64-partition halves so each head's score
  matmul finds its operands on matching partition bases.
- scores are built transposed ([k, q]); v carries a trailing ones column so
  the PV accumulation yields unnormalized outT plus the softmax row sums in
  one PSUM tile; normalization is reciprocal + a K=1 broadcast matmul + one
  multiply.
- Causal masking: strictly-upper score blocks are skipped; the 4 partially
  masked diagonal strips are multiplied by exp(mask) strips (exact: the
  reference adds mask pre-softmax, and exp(-1e9) underflows to 0 either way).
- Matmuls run as float32r (fp32 data, full-rate PE mode for free dim >= 256).
"""

import numpy as np

import concourse.bass as bass
import concourse.mybir as mybir
import concourse.tile as tile
from concourse.bass_utils import run_bass_kernel_spmd

B, S, D = 2, 2048, 1024
H, KVH, HD = 16, 4, 64
GH = H // KVH  # 4 q heads per core
SC = 512  # s-chunk
NCH = S // SC  # 4 chunks
DT = D // 128  # 8 d-tiles
F32 = mybir.dt.float32
F32R = mybir.dt.float32r


def build_nc(split=True):
    nc = bass.Bass("TRN2", target_bir_lowering=False, debug=False, num_devices=1)

    xt = nc.dram_tensor("xt", [D, S], F32R, kind="ExternalInput").ap()
    # [deint (4h x 64) | swap (4h x 64)]
    wq = nc.dram_tensor("wq", [D, 2 * GH * HD], F32R, kind="ExternalInput").ap()
    # [k_deint | k_deint | k_swap | k_swap]
    wk2 = nc.dram_tensor("wk2", [D, 4 * HD], F32R, kind="ExternalInput").ap()
    wv = nc.dram_tensor("wv", [D, HD], F32R, kind="ExternalInput").ap()
    wo = nc.dram_tensor("wo", [GH * HD, D], F32R, kind="ExternalInput").ap()
    cos4 = nc.dram_tensor("cos4", [128, S], F32, kind="ExternalInput").ap()
    sin4 = nc.dram_tensor("sin4", [128, S], F32, kind="ExternalInput").ap()
    m01 = nc.dram_tensor("m01", [4, 128, SC], F32R, kind="ExternalInput").ap()
    ident = nc.dram_tensor("ident", [128, 128], F32R, kind="ExternalInput").ap()
    onesd = nc.dram_tensor("onesd", [128, 128], F32R, kind="ExternalInput").ap()
    out = nc.dram_tensor("out", [S, D], F32, kind="ExternalOutput").ap()

    with tile.TileContext(nc) as tc:
        from contextlib import ExitStack

        with ExitStack() as ctx:
            singles = ctx.enter_context(tc.tile_pool(name="singles", bufs=1))
            persist = ctx.enter_context(tc.tile_pool(name="persist", bufs=1))
            xt_pool = ctx.enter_context(tc.tile_pool(name="xt", bufs=2))
            tmp_pool = ctx.enter_context(tc.tile_pool(name="tmp", bufs=2))
            probs_pool = ctx.enter_context(tc.tile_pool(name="probs", bufs=4))
            rec_pool = ctx.enter_context(tc.tile_pool(name="rec", bufs=2))
            rb_pool = ctx.enter_context(tc.tile_pool(name="rb", bufs=2))
            odd_pool = ctx.enter_context(tc.tile_pool(name="odd", bufs=2))
            stage_pool = ctx.enter_context(tc.tile_pool(name="stage", bufs=2))
            psA = ctx.enter_context(tc.tile_pool(name="psA", bufs=4, space="PSUM"))
            psB = ctx.enter_context(tc.tile_pool(name="psB", bufs=2, space="PSUM"))
            psC = ctx.enter_context(tc.tile_pool(name="psC", bufs=2, space="PSUM"))

            # ---- constants ----
            wq_sb = []
            wk2_sb = []
            wv_sb = []
            for dt in range(DT):
                ds = slice(128 * dt, 128 * (dt + 1))
                t = singles.tile([128, 2 * GH * HD], F32R, tag=f"wq{dt}", name=f"wq{dt}")
                nc.sync.dma_start(out=t, in_=wq[ds, :])
                wq_sb.append(t)
                t = singles.tile([128, 4 * HD], F32R, tag=f"wk{dt}", name=f"wk{dt}")
                nc.sync.dma_start(out=t, in_=wk2[ds, :])
                wk2_sb.append(t)
                t = singles.tile([128, HD], F32R, tag=f"wv{dt}", name=f"wv{dt}")
                nc.sync.dma_start(out=t, in_=wv[ds, :])
                wv_sb.append(t)
            wo_sb = []
            for r in range(2):
                t = singles.tile([128, D], F32R, tag=f"wo{r}", name=f"wo{r}")
                nc.sync.dma_start(out=t, in_=wo[128 * r : 128 * (r + 1), :])
                wo_sb.append(t)
            cos_sb = singles.tile([128, S], F32, tag="cos")
            nc.sync.dma_start(out=cos_sb, in_=cos4)
            sin_sb = singles.tile([128, S], F32, tag="sin")
            nc.sync.dma_start(out=sin_sb, in_=sin4)
            m01_sb = []
            for j in range(4):
                t = singles.tile([128, SC], F32R, tag=f"m01{j}", name=f"m01{j}")
                nc.sync.dma_start(out=t, in_=m01[j])
                m01_sb.append(t)
            ident_sb = singles.tile([128, 128], F32R, tag="ident")
            nc.sync.dma_start(out=ident_sb, in_=ident)
            # ones row living on partition 64 (for the K=1 broadcast matmul
            # against the row-64 softmax sums)
            ones_t = singles.tile([65, 128], F32R, tag="ones")
            nc.sync.dma_start(out=ones_t[64:65, :], in_=onesd[0:1, :])

            # ---- persistent activations ----
            qrot = [
                persist.tile([128, S], F32R, tag=f"qrot{p}", name=f"qrot{p}")
                for p in range(2)
            ]
            krot2 = persist.tile([128, S], F32R, tag="krot2")
            v_sb = [
                persist.tile([128, 65], F32R, tag=f"v{kb}", name=f"v{kb}")
                for kb in range(S // 128)
            ]
            outT = [
                persist.tile([128, S], F32R, tag=f"outT{r}", name=f"outT{r}")
                for r in range(2)
            ]

            for c in range(NCH):
                cs = slice(SC * c, SC * (c + 1))

                xt_sb = []
                for dt in range(DT):
                    t = xt_pool.tile([128, SC], F32R, tag=f"xt{dt}", name=f"xt{dt}")
                    nc.sync.dma_start(out=t, in_=xt[128 * dt : 128 * (dt + 1), cs])
                    xt_sb.append(t)

                # ---- q projection + rope (2 head-pairs) ----
                for p in range(2):
                    qd_ps = psA.tile([128, SC], F32, tag="mm", name="qd_ps")
                    qs_ps = psA.tile([128, SC], F32, tag="mm", name="qs_ps")
                    for dt in range(DT):
                        nc.tensor.matmul(
                            qd_ps,
                            (wq_sb[dt][:, 128 * p : 128 * (p + 1)]),
                            (xt_sb[dt]),
                            start=(dt == 0),
                            stop=(dt == DT - 1),
                        )
                    for dt in range(DT):
                        nc.tensor.matmul(
                            qs_ps,
                            (wq_sb[dt][:, 256 + 128 * p : 256 + 128 * (p + 1)]),
                            (xt_sb[dt]),
                            start=(dt == 0),
                            stop=(dt == DT - 1),
                        )
                    tm1 = tmp_pool.tile([128, SC], F32, tag="tm1")
                    tm2 = tmp_pool.tile([128, SC], F32, tag="tm2")
                    nc.vector.tensor_tensor(
                        tm1, qd_ps, cos_sb[:, cs], mybir.AluOpType.mult
                    )
                    nc.vector.tensor_tensor(
                        tm2, qs_ps, sin_sb[:, cs], mybir.AluOpType.mult
                    )
                    nc.vector.tensor_tensor(
                        qrot[p][:, cs], tm1, tm2, mybir.AluOpType.add
                    )

                # ---- k projection (duplicated halves) + rope ----
                kd_ps = psA.tile([128, SC], F32, tag="mm", name="kd_ps")
                ks_ps = psA.tile([128, SC], F32, tag="mm", name="ks_ps")
                for dt in range(DT):
                    nc.tensor.matmul(
                        kd_ps,
                        (wk2_sb[dt][:, 0:128]),
                        (xt_sb[dt]),
                        start=(dt == 0),
                        stop=(dt == DT - 1),
                    )
                for dt in range(DT):
                    nc.tensor.matmul(
                        ks_ps,
                        (wk2_sb[dt][:, 128:256]),
                        (xt_sb[dt]),
                        start=(dt == 0),
                        stop=(dt == DT - 1),
                    )
                km1 = tmp_pool.tile([128, SC], F32, tag="tm1")
                km2 = tmp_pool.tile([128, SC], F32, tag="tm2")
                nc.vector.tensor_tensor(km1, kd_ps, cos_sb[:, cs], mybir.AluOpType.mult)
                nc.vector.tensor_tensor(km2, ks_ps, sin_sb[:, cs], mybir.AluOpType.mult)
                nc.vector.tensor_tensor(krot2[:, cs], km1, km2, mybir.AluOpType.add)

                # ---- v projection + transpose into [k, hd] tiles ----
                v_ps = psC.tile([64, SC], F32, tag="misc", name="v_ps")
                for dt in range(DT):
                    nc.tensor.matmul(
                        v_ps,
                        (wv_sb[dt]),
                        (xt_sb[dt]),
                        start=(dt == 0),
                        stop=(dt == DT - 1),
                    )
                vT = tmp_pool.tile([64, SC], F32R, tag="vT")
                nc.vector.tensor_copy(vT, v_ps)
                for j in range(4):
                    kb = 4 * c + j
                    vt_ps = psC.tile([128, 64], F32R, tag="misc", name="vt_ps")
                    nc.tensor.transpose(
                        vt_ps, vT[:, 128 * j : 128 * (j + 1)], ident_sb[0:64, 0:64]
                    )
                    nc.vector.tensor_copy(v_sb[kb][:, 0:64], vt_ps)
                    nc.sync.dma_start(out=v_sb[kb][:, 64:65], in_=onesd[:, 0:1])

                # ---- attention per head ----
                for h in range(GH):
                    p, hh = divmod(h, 2)
                    hb = slice(64 * hh, 64 * (hh + 1))
                    o_ps = psB.tile([65, SC], F32, tag="o", name="o_ps")
                    nkb = 4 * c + 4
                    for kb in range(nkb):
                        s_ps = psA.tile([128, SC], F32, tag="mm", name="s_ps")
                        nc.tensor.matmul(
                            s_ps,
                            (krot2[hb, 128 * kb : 128 * (kb + 1)]),
                            (qrot[p][hb, cs]),
                            start=True,
                            stop=True,
                        )
                        pt = probs_pool.tile([128, SC], F32R, tag="pt", name="pt")
                        nc.scalar.activation(
                            pt, s_ps, mybir.ActivationFunctionType.Exp, scale=0.125
                        )
                        if kb >= 4 * c:
                            j = kb - 4 * c
                            w = 128 * (j + 1)
                            nc.gpsimd.tensor_tensor(
                                pt[:, 0:w],
                                pt[:, 0:w],
                                m01_sb[j][:, 0:w],
                                mybir.AluOpType.mult,
                            )
                        nc.tensor.matmul(
                            o_ps,
                            (v_sb[kb][:, 0:65]),
                            (pt),
                            start=(kb == 0),
                            stop=(kb == nkb - 1),
                        )
                    # normalize -> outT (pair tile, head hh half)
                    rec = rec_pool.tile([65, SC], F32R, tag="rec", name="rec")
                    with nc.allow_low_precision(reason="softmax denom in f32r"):
                        nc.vector.reciprocal(rec[64:65], o_ps[64:65])
                    rb_ps = psC.tile([128, SC], F32, tag="misc", name="rb_ps")
                    nc.tensor.matmul(
                        rb_ps,
                        (ones_t[64:65, :]),
                        (rec[64:65]),
                        start=True,
                        stop=True,
                    )
                    rb_sb = rb_pool.tile([128, SC], F32, tag="rb", name="rb_sb")
                    nc.vector.tensor_copy(rb_sb, rb_ps)
                    if hh == 0:
                        nc.vector.tensor_tensor(
                            outT[p][0:64, cs],
                            o_ps[0:64],
                            rb_sb[0:64],
                            mybir.AluOpType.mult,
                        )
                    else:
                        odd_t = odd_pool.tile([64, SC], F32R, tag="odd", name="odd_t")
                        nc.vector.tensor_tensor(
                            odd_t, o_ps[0:64], rb_sb[0:64], mybir.AluOpType.mult
                        )
                        # partition-shifting copy must go through DMA
                        nc.sync.dma_start(out=outT[p][64:128, cs], in_=odd_t)

                # ---- output projection partial for this chunk ----
                for sb_i in range(4):
                    r0 = SC * c + 128 * sb_i
                    stage = stage_pool.tile([128, D], F32, tag="stage", name="stage")
                    for n in range(2):
                        w_ps = psA.tile([128, SC], F32, tag="mm", name="w_ps")
                        for r in range(2):
                            nc.tensor.matmul(
                                w_ps,
                                (outT[r][:, r0 : r0 + 128]),
                                (wo_sb[r][:, SC * n : SC * (n + 1)]),
                                start=(r == 0),
                                stop=(r == 1),
                            )
                        nc.scalar.copy(stage[:, SC * n : SC * (n + 1)], w_ps)
                    nc.sync.dma_start(out=out[r0 : r0 + 128, :], in_=stage)

    if split:
        split_excess_waits(nc)
    return nc


def split_excess_waits(nc, max_waits=1):
    """This container's walrus codegen supports one semaphore wait per
    instruction. Hoist excess waits onto NOPs injected just before, on the
    same engine (engine program order preserves the semantics)."""
    n_split = 0
    for fn in nc.m.functions:
        for bb in fn.blocks:
            insts = bb.instructions
            new = []
            for inst in insts:
                si = inst.sync_info
                waits = list(si.on_wait) if si is not None and si.on_wait else []
                if len(waits) > max_waits:
                    n_split += 1
                    extra, keep = waits[:-max_waits], waits[-max_waits:]
                    for k in range(0, len(extra), max_waits):
                        nop = mybir.InstNoOp(
                            name=nc.get_next_instruction_name(), ins=[], outs=[]
                        )
                        nop.engine = inst.engine
                        nop.sync_info = mybir.SyncInfo(
                            on_wait=extra[k : k + max_waits], on_update=[]
                        )
                        new.append(nop)
                    inst.sync_info = mybir.SyncInfo(
                        on_wait=keep,
                        on_update=list(si.on_update) if si.on_update else [],
                    )
                new.append(inst)
            bb.instructions = new
    return n_split


def _host_shards(x, wq, wk, wv, wo, freqs_cos, freqs_sin, mask):
    deint = np.concatenate([np.arange(0, HD, 2), np.arange(1, HD, 2)])
    # swap columns: col j (j<32) = -orig[2j+1]; col 32+j = orig[2j]
    swap_idx = np.concatenate([np.arange(1, HD, 2), np.arange(0, HD, 2)])
    swap_sign = np.concatenate(
        [-np.ones(HD // 2, np.float32), np.ones(HD // 2, np.float32)]
    )

    cos4 = np.ascontiguousarray(np.tile(freqs_cos.T, (4, 1)), dtype=np.float32)
    sin4 = np.ascontiguousarray(np.tile(freqs_sin.T, (4, 1)), dtype=np.float32)
    # exp(mask) strips for the 4 diagonal k-blocks of each 512-wide q chunk
    m01 = np.empty((4, 128, SC), dtype=np.float32)
    mblk = np.asarray(mask[0, 0, 0:SC, 0:SC], dtype=np.float64)
    for j in range(4):
        m01[j] = np.exp(mblk[:, 128 * j : 128 * (j + 1)].T).astype(np.float32)
    ident = np.eye(128, dtype=np.float32)

    xts = [np.ascontiguousarray(x[b].T, dtype=np.float32) for b in range(B)]

    in_maps = []
    for core in range(8):
        b, g = divmod(core, KVH)
        wq_g = wq[:, g * GH * HD : (g + 1) * GH * HD]
        wq_d = np.concatenate([wq_g[:, h * HD + deint] for h in range(GH)], axis=1)
        wq_s = np.concatenate(
            [wq_g[:, h * HD + swap_idx] * swap_sign for h in range(GH)], axis=1
        )
        wq2 = np.ascontiguousarray(
            np.concatenate([wq_d, wq_s], axis=1), dtype=np.float32
        )
        wk_g = wk[:, g * HD : (g + 1) * HD]
        wk_d = wk_g[:, deint]
        wk_s = wk_g[:, swap_idx] * swap_sign
        wk2 = np.ascontiguousarray(
            np.concatenate([wk_d, wk_d, wk_s, wk_s], axis=1), dtype=np.float32
        )
        wv_g = np.ascontiguousarray(wv[:, g * HD : (g + 1) * HD], dtype=np.float32)
        wo_g = np.ascontiguousarray(
            wo[g * GH * HD : (g + 1) * GH * HD, :], dtype=np.float32
        )
        in_maps.append(
            {
                "xt": xts[b],
                "wq": wq2,
                "wk2": wk2,
                "wv": wv_g,
                "wo": wo_g,
                "cos4": cos4,
                "sin4": sin4,
                "m01": m01,
                "ident": ident,
                "onesd": np.ones((128, 128), dtype=np.float32),
            }
        )
    return in_maps


_NC_CACHE = None


def get_nc():
    global _NC_CACHE
    if _NC_CACHE is None:
        _NC_CACHE = build_nc()
    return _NC_CACHE


def kernel(x, wq, wk, wv, wo, freqs_cos, freqs_sin, mask):
    in_maps = _host_shards(
        np.asarray(x),
        np.asarray(wq),
        np.asarray(wk),
        np.asarray(wv),
        np.asarray(wo),
        np.asarray(freqs_cos),
        np.asarray(freqs_sin),
        np.asarray(mask),
    )
    nc = get_nc()
    res = run_bass_kernel_spmd(nc, in_maps, core_ids=list(range(8)))
    parts = [res.results[i]["out"] for i in range(8)]
    out = np.stack(
        [
            parts[0] + parts[1] + parts[2] + parts[3],
            parts[4] + parts[5] + parts[6] + parts[7],
        ]
    ).astype(np.float32)
    return out


# revision 17
# speedup vs baseline: 1.2321x; 1.2321x over previous
"""GQA causal attention block (B=2, S=2048, D=1024, H=16, KVH=4) on 8 trn2
NeuronCores.

Sharding: core = (b, g) for batch b in {0,1} and kv-head group g in {0..3}.
Each core projects q for its 4 query heads (column-parallel wq), k/v for its
kv head, applies RoPE, runs causal attention for its 4 heads, and computes a
row-parallel partial of the output projection (its 256 rows of wo). The host
sums the 4 partials per batch.

Device layout choices:
- x is shipped pre-transposed (xt = x[b].T) so the D contraction sits on
  SBUF partitions for every projection matmul.
- RoPE head dims are de-interleaved (even dims then odd dims) via a host-side
  permutation of wq/wk columns, and the rotation partner comes from a second
  matmul against sign-flipped swapped columns:
      q_rot = cos * (x @ wq_deint) + sin * (x @ wq_swap)
  which keeps every vector op partition-aligned (the hardware requires all
  elementwise operands on identical partitions). Scores are invariant to the
  shared permutation of q and k head dims.
- q heads are produced in pair tiles (two heads stacked on 128 partitions);
  k is produced duplicated on both 64-partition halves so each head's score
  matmul finds its operands on matching partition bases.
- scores are built transposed ([k, q]); v carries a trailing ones column so
  the PV accumulation yields unnormalized outT plus the softmax row sums in
  one PSUM tile; normalization is reciprocal + a partition-broadcast DMA +
  one multiply, software-pipelined one head behind the attention loop.
- Causal masking: strictly-upper score blocks are skipped; diagonal blocks
  are narrowed to their valid columns, and the one triangular 128x128
  sub-block is multiplied by exp(mask) strips (exact: the reference adds the
  mask pre-softmax, and exp(-1e9) underflows to 0 either way).
- Matmuls run as float32r (fp32 data, full-rate PE mode for free dim >= 256).
"""

import numpy as np

import concourse.bass as bass
import concourse.mybir as mybir
import concourse.tile as tile
from concourse.bass_utils import run_bass_kernel_spmd

B, S, D = 2, 2048, 1024
H, KVH, HD = 16, 4, 64
GH = H // KVH  # 4 q heads per core
SC = 512  # s-chunk
NCH = S // SC  # 4 chunks
DT = D // 128  # 8 d-tiles
F32 = mybir.dt.float32
F32R = mybir.dt.float32r


def build_nc(split=True):
    nc = bass.Bass("TRN2", target_bir_lowering=False, debug=False, num_devices=1)

    xt = nc.dram_tensor("xt", [D, S], F32R, kind="ExternalInput").ap()
    # [deint (4h x 64) | swap (4h x 64)]
    wq = nc.dram_tensor("wq", [D, 2 * GH * HD], F32R, kind="ExternalInput").ap()
    # [k_deint | k_deint | k_swap | k_swap]
    wk2 = nc.dram_tensor("wk2", [D, 4 * HD], F32R, kind="ExternalInput").ap()
    wv = nc.dram_tensor("wv", [D, HD], F32R, kind="ExternalInput").ap()
    wo = nc.dram_tensor("wo", [GH * HD, D], F32R, kind="ExternalInput").ap()
    cos4 = nc.dram_tensor("cos4", [128, S], F32, kind="ExternalInput").ap()
    sin4 = nc.dram_tensor("sin4", [128, S], F32, kind="ExternalInput").ap()
    m01 = nc.dram_tensor("m01", [4, 128, SC], F32R, kind="ExternalInput").ap()
    ident = nc.dram_tensor("ident", [128, 128], F32R, kind="ExternalInput").ap()
    onesd = nc.dram_tensor("onesd", [128, 128], F32R, kind="ExternalInput").ap()
    out = nc.dram_tensor("out", [S, D], F32, kind="ExternalOutput").ap()

    with tile.TileContext(nc) as tc:
        from contextlib import ExitStack

        with ExitStack() as ctx:
            singles = ctx.enter_context(tc.tile_pool(name="singles", bufs=1))
            persist = ctx.enter_context(tc.tile_pool(name="persist", bufs=1))
            xt_pool = ctx.enter_context(tc.tile_pool(name="xt", bufs=2))
            tmp_pool = ctx.enter_context(tc.tile_pool(name="tmp", bufs=2))
            probs_pool = ctx.enter_context(tc.tile_pool(name="probs", bufs=8))
            rec_pool = ctx.enter_context(tc.tile_pool(name="rec", bufs=2))
            rb_pool = ctx.enter_context(tc.tile_pool(name="rb", bufs=2))
            odd_pool = ctx.enter_context(tc.tile_pool(name="odd", bufs=2))
            stage_pool = ctx.enter_context(tc.tile_pool(name="stage", bufs=2))
            psA = ctx.enter_context(tc.tile_pool(name="psA", bufs=3, space="PSUM"))
            psQ = ctx.enter_context(tc.tile_pool(name="psQ", bufs=2, space="PSUM"))
            psB = ctx.enter_context(tc.tile_pool(name="psB", bufs=3, space="PSUM"))
            dscr = ctx.enter_context(tc.tile_pool(name="dscr", bufs=2, space="DRAM"))

            # ---- constants + first x chunk (DMA order = need order) ----
            wq_sb = []
            xt_sb0 = []
            for dt in range(DT):
                ds = slice(128 * dt, 128 * (dt + 1))
                t = singles.tile([128, 2 * GH * HD], F32R, tag=f"wq{dt}", name=f"wq{dt}")
                nc.sync.dma_start(out=t, in_=wq[ds, :])
                wq_sb.append(t)
                t = xt_pool.tile([128, SC], F32R, tag=f"xt{dt}", name=f"xt{dt}")
                nc.sync.dma_start(out=t, in_=xt[ds, 0:SC])
                xt_sb0.append(t)
            cos_sb = singles.tile([128, S], F32, tag="cos")
            nc.sync.dma_start(out=cos_sb, in_=cos4)
            sin_sb = singles.tile([128, S], F32, tag="sin")
            nc.sync.dma_start(out=sin_sb, in_=sin4)
            wk2_sb = []
            wv_sb = []
            for dt in range(DT):
                ds = slice(128 * dt, 128 * (dt + 1))
                t = singles.tile([128, 4 * HD], F32R, tag=f"wk{dt}", name=f"wk{dt}")
                nc.sync.dma_start(out=t, in_=wk2[ds, :])
                wk2_sb.append(t)
                t = singles.tile([128, HD], F32R, tag=f"wv{dt}", name=f"wv{dt}")
                nc.sync.dma_start(out=t, in_=wv[ds, :])
                wv_sb.append(t)
            m01_sb = []
            for j in range(4):
                t = singles.tile([128, SC], F32R, tag=f"m01{j}", name=f"m01{j}")
                nc.sync.dma_start(out=t, in_=m01[j])
                m01_sb.append(t)
            ident_sb = singles.tile([128, 128], F32R, tag="ident")
            nc.sync.dma_start(out=ident_sb, in_=ident)
            wo_sb = []
            for r in range(2):
                t = singles.tile([128, D], F32R, tag=f"wo{r}", name=f"wo{r}")
                nc.sync.dma_start(out=t, in_=wo[128 * r : 128 * (r + 1), :])
                wo_sb.append(t)

            # ---- persistent activations ----
            # qrot: per (pair, chunk) pool tiles, double-buffered across chunks
            qrot_pool = ctx.enter_context(tc.tile_pool(name="qrotp", bufs=2))
            # k and v persist across chunks (they are the KV cache)
            krot_c = [
                persist.tile([128, SC], F32R, tag=f"krot{c}", name=f"krot{c}")
                for c in range(NCH)
            ]
            v_sb = [
                persist.tile([128, 65], F32R, tag=f"v{kb}", name=f"v{kb}")
                for kb in range(S // 128)
            ]
            outT_pool = ctx.enter_context(tc.tile_pool(name="outTp", bufs=2))

            def emit_norm(p, hh, o_ps, outT_c):
                rec = rec_pool.tile([65, SC], F32, tag="rec", name="rec")
                with nc.allow_low_precision(reason="softmax denom"):
                    nc.vector.reciprocal(rec[64:65], o_ps[64:65])
                rd = dscr.tile([1, SC], F32, tag="rd", name="rd")
                nc.scalar.dma_start(out=rd, in_=rec[64:65])
                rb_sb = rb_pool.tile([64, SC], F32, tag="rb", name="rb_sb")
                nc.scalar.dma_start(out=rb_sb, in_=rd.to_broadcast((64, SC)))
                if hh == 0:
                    nc.vector.tensor_tensor(
                        outT_c[p][0:64, :], o_ps[0:64], rb_sb, mybir.AluOpType.mult
                    )
                else:
                    odd_t = odd_pool.tile([64, SC], F32R, tag="odd", name="odd_t")
                    nc.vector.tensor_tensor(
                        odd_t, o_ps[0:64], rb_sb, mybir.AluOpType.mult
                    )
                    # partition-shifting copy must go through DMA
                    nc.gpsimd.dma_start(out=outT_c[p][64:128, :], in_=odd_t)

            def emit_wo(c, outT_c):
                for sb_i in range(4):
                    r0 = SC * c + 128 * sb_i
                    stage = stage_pool.tile([128, D], F32, tag="stage", name="stage")
                    for n in range(2):
                        w_ps = psA.tile([128, SC], F32, tag="mm", name="w_ps")
                        for r in range(2):
                            nc.tensor.matmul(
                                w_ps,
                                outT_c[r][:, 128 * sb_i : 128 * (sb_i + 1)],
                                wo_sb[r][:, SC * n : SC * (n + 1)],
                                start=(r == 0),
                                stop=(r == 1),
                            )
                        nc.vector.tensor_copy(stage[:, SC * n : SC * (n + 1)], w_ps)
                    nc.sync.dma_start(out=out[r0 : r0 + 128, :], in_=stage)

            pending_wo = None
            for c in range(NCH):
                cs = slice(SC * c, SC * (c + 1))

                if c == 0:
                    xt_sb = xt_sb0
                else:
                    xt_sb = []
                    for dt in range(DT):
                        t = xt_pool.tile([128, SC], F32R, tag=f"xt{dt}", name=f"xt{dt}")
                        nc.sync.dma_start(out=t, in_=xt[128 * dt : 128 * (dt + 1), cs])
                        xt_sb.append(t)

                qrot_c = [
                    qrot_pool.tile([128, SC], F32R, tag=f"qrot{p}", name=f"qrot{p}")
                    for p in range(2)
                ]
                outT_c = [
                    outT_pool.tile([128, SC], F32R, tag=f"outT{r}", name=f"outT{r}")
                    for r in range(2)
                ]

                # ---- q projection + rope (2 head-pairs) ----
                for p in range(2):
                    qd_ps = psQ.tile([128, SC], F32, tag="qk", name="qd_ps")
                    qs_ps = psQ.tile([128, SC], F32, tag="qk", name="qs_ps")
                    for dt in range(DT):
                        nc.tensor.matmul(
                            qd_ps,
                            wq_sb[dt][:, 128 * p : 128 * (p + 1)],
                            xt_sb[dt],
                            start=(dt == 0),
                            stop=(dt == DT - 1),
                        )
                    for dt in range(DT):
                        nc.tensor.matmul(
                            qs_ps,
                            wq_sb[dt][:, 256 + 128 * p : 256 + 128 * (p + 1)],
                            xt_sb[dt],
                            start=(dt == 0),
                            stop=(dt == DT - 1),
                        )
                    tm1 = tmp_pool.tile([128, SC], F32, tag="tm1")
                    tm2 = tmp_pool.tile([128, SC], F32, tag="tm2")
                    nc.vector.tensor_tensor(
                        tm1, qd_ps, cos_sb[:, cs], mybir.AluOpType.mult
                    )
                    nc.vector.tensor_tensor(
                        tm2, qs_ps, sin_sb[:, cs], mybir.AluOpType.mult
                    )
                    nc.vector.tensor_tensor(
                        qrot_c[p], tm1, tm2, mybir.AluOpType.add
                    )

                # ---- k projection (duplicated halves) + rope ----
                kd_ps = psQ.tile([128, SC], F32, tag="qk", name="kd_ps")
                ks_ps = psQ.tile([128, SC], F32, tag="qk", name="ks_ps")
                for dt in range(DT):
                    nc.tensor.matmul(
                        kd_ps,
                        wk2_sb[dt][:, 0:128],
                        xt_sb[dt],
                        start=(dt == 0),
                        stop=(dt == DT - 1),
                    )
                for dt in range(DT):
                    nc.tensor.matmul(
                        ks_ps,
                        wk2_sb[dt][:, 128:256],
                        xt_sb[dt],
                        start=(dt == 0),
                        stop=(dt == DT - 1),
                    )
                km1 = tmp_pool.tile([128, SC], F32, tag="tm1")
                km2 = tmp_pool.tile([128, SC], F32, tag="tm2")
                nc.vector.tensor_tensor(km1, kd_ps, cos_sb[:, cs], mybir.AluOpType.mult)
                nc.vector.tensor_tensor(km2, ks_ps, sin_sb[:, cs], mybir.AluOpType.mult)
                nc.vector.tensor_tensor(krot_c[c], km1, km2, mybir.AluOpType.add)

                # ---- v projection + transpose into [k, hd] tiles ----
                v_ps = psQ.tile([64, SC], F32, tag="qk", name="v_ps")
                for dt in range(DT):
                    nc.tensor.matmul(
                        v_ps,
                        wv_sb[dt],
                        xt_sb[dt],
                        start=(dt == 0),
                        stop=(dt == DT - 1),
                    )
                vT = tmp_pool.tile([64, SC], F32R, tag="vT")
                nc.vector.tensor_copy(vT, v_ps)
                for j in range(4):
                    kb = 4 * c + j
                    vt_ps = psQ.tile([128, 64], F32R, tag="qk", name="vt_ps")
                    nc.tensor.transpose(
                        vt_ps, vT[:, 128 * j : 128 * (j + 1)], ident_sb[0:64, 0:64]
                    )
                    nc.vector.tensor_copy(v_sb[kb][:, 0:64], vt_ps)
                    nc.gpsimd.dma_start(out=v_sb[kb][:, 64:65], in_=onesd[:, 0:1])

                # ---- attention: two heads interleaved per pass ----
                nkb = 4 * c + 4
                for hp in range(2):
                    heads = [2 * hp, 2 * hp + 1]
                    o_pss = {}
                    for h in heads:
                        o_pss[h] = psB.tile([65, SC], F32, tag="o", name="o_ps")
                    for kb in range(nkb):
                        j = kb - 4 * c
                        col0 = 128 * j if j >= 0 else 0
                        for h in heads:
                            p, hh = divmod(h, 2)
                            hb = slice(64 * hh, 64 * (hh + 1))
                            s_ps = psA.tile([128, SC], F32, tag="mm", name="s_ps")
                            nc.tensor.matmul(
                                s_ps[:, col0:],
                                krot_c[kb // 4][hb, 128 * (kb % 4) : 128 * (kb % 4 + 1)],
                                qrot_c[p][hb, col0:],
                                start=True,
                                stop=True,
                            )
                            pt = probs_pool.tile([128, SC], F32R, tag="pt", name="pt")
                            nc.scalar.activation(
                                pt[:, col0:],
                                s_ps[:, col0:],
                                mybir.ActivationFunctionType.Exp,
                                scale=0.125,
                            )
                            if j >= 0:
                                nc.gpsimd.tensor_tensor(
                                    pt[:, col0 : col0 + 128],
                                    pt[:, col0 : col0 + 128],
                                    m01_sb[j][:, col0 : col0 + 128],
                                    mybir.AluOpType.mult,
                                )
                            nc.tensor.matmul(
                                o_pss[h][:, col0:],
                                v_sb[kb][:, 0:65],
                                pt[:, col0:],
                                start=(kb == 0),
                                stop=(kb == nkb - 1),
                            )
                    for h in heads:
                        p, hh = divmod(h, 2)
                        emit_norm(p, hh, o_pss[h], outT_c)

                # ---- output projection pipelined one chunk behind ----
                if pending_wo is not None:
                    emit_wo(*pending_wo)
                pending_wo = (c, outT_c)
            emit_wo(*pending_wo)

    if split:
        split_excess_waits(nc)
    return nc


def split_excess_waits(nc, max_waits=1):
    """This container's walrus codegen supports one semaphore wait per
    instruction. Hoist excess waits onto NOPs injected just before, on the
    same engine (engine program order preserves the semantics)."""
    n_split = 0
    for fn in nc.m.functions:
        for bb in fn.blocks:
            insts = bb.instructions
            new = []
            for inst in insts:
                si = inst.sync_info
                waits = list(si.on_wait) if si is not None and si.on_wait else []
                if len(waits) > max_waits:
                    n_split += 1
                    extra, keep = waits[:-max_waits], waits[-max_waits:]
                    for k in range(0, len(extra), max_waits):
                        nop = mybir.InstNoOp(
                            name=nc.get_next_instruction_name(), ins=[], outs=[]
                        )
                        nop.engine = inst.engine
                        nop.sync_info = mybir.SyncInfo(
                            on_wait=extra[k : k + max_waits], on_update=[]
                        )
                        new.append(nop)
                    inst.sync_info = mybir.SyncInfo(
                        on_wait=keep,
                        on_update=list(si.on_update) if si.on_update else [],
                    )
                new.append(inst)
            bb.instructions = new
    return n_split


def _host_shards(x, wq, wk, wv, wo, freqs_cos, freqs_sin, mask):
    deint = np.concatenate([np.arange(0, HD, 2), np.arange(1, HD, 2)])
    # swap columns: col j (j<32) = -orig[2j+1]; col 32+j = orig[2j]
    swap_idx = np.concatenate([np.arange(1, HD, 2), np.arange(0, HD, 2)])
    swap_sign = np.concatenate(
        [-np.ones(HD // 2, np.float32), np.ones(HD // 2, np.float32)]
    )

    cos4 = np.ascontiguousarray(np.tile(freqs_cos.T, (4, 1)), dtype=np.float32)
    sin4 = np.ascontiguousarray(np.tile(freqs_sin.T, (4, 1)), dtype=np.float32)
    # exp(mask) strips for the 4 diagonal k-blocks of each 512-wide q chunk
    m01 = np.empty((4, 128, SC), dtype=np.float32)
    mblk = np.asarray(mask[0, 0, 0:SC, 0:SC], dtype=np.float64)
    for j in range(4):
        m01[j] = np.exp(mblk[:, 128 * j : 128 * (j + 1)].T).astype(np.float32)
    ident = np.eye(128, dtype=np.float32)

    xts = [np.ascontiguousarray(x[b].T, dtype=np.float32) for b in range(B)]

    in_maps = []
    for core in range(8):
        b, g = divmod(core, KVH)
        wq_g = wq[:, g * GH * HD : (g + 1) * GH * HD]
        wq_d = np.concatenate([wq_g[:, h * HD + deint] for h in range(GH)], axis=1)
        wq_s = np.concatenate(
            [wq_g[:, h * HD + swap_idx] * swap_sign for h in range(GH)], axis=1
        )
        wq2 = np.ascontiguousarray(
            np.concatenate([wq_d, wq_s], axis=1), dtype=np.float32
        )
        wk_g = wk[:, g * HD : (g + 1) * HD]
        wk_d = wk_g[:, deint]
        wk_s = wk_g[:, swap_idx] * swap_sign
        wk2a = np.ascontiguousarray(
            np.concatenate([wk_d, wk_d, wk_s, wk_s], axis=1), dtype=np.float32
        )
        wv_g = np.ascontiguousarray(wv[:, g * HD : (g + 1) * HD], dtype=np.float32)
        wo_g = np.ascontiguousarray(
            wo[g * GH * HD : (g + 1) * GH * HD, :], dtype=np.float32
        )
        in_maps.append(
            {
                "xt": xts[b],
                "wq": wq2,
                "wk2": wk2a,
                "wv": wv_g,
                "wo": wo_g,
                "cos4": cos4,
                "sin4": sin4,
                "m01": m01,
                "ident": ident,
                "onesd": np.ones((128, 128), dtype=np.float32),
            }
        )
    return in_maps


_NC_CACHE = None


def get_nc():
    global _NC_CACHE
    if _NC_CACHE is None:
        _NC_CACHE = build_nc()
    return _NC_CACHE


def kernel(x, wq, wk, wv, wo, freqs_cos, freqs_sin, mask):
    in_maps = _host_shards(
        np.asarray(x),
        np.asarray(wq),
        np.asarray(wk),
        np.asarray(wv),
        np.asarray(wo),
        np.asarray(freqs_cos),
        np.asarray(freqs_sin),
        np.asarray(mask),
    )
    nc = get_nc()
    res = run_bass_kernel_spmd(nc, in_maps, core_ids=list(range(8)))
    parts = [res.results[i]["out"] for i in range(8)]
    out = np.stack(
        [
            parts[0] + parts[1] + parts[2] + parts[3],
            parts[4] + parts[5] + parts[6] + parts[7],
        ]
    ).astype(np.float32)
    return out
